# revision 30
# baseline (speedup 1.0000x reference)
"""Trainium2 Bass kernel for nn_ByteLevelDecoder.

Data-parallel over 512 byte streams, 64 per core on 8 cores. v2 design:
- Single ACT table set (gelu_and_others) for the whole kernel: softmax
  exp is computed from tanh (exp(s) = (1+tanh(s/2))/(1-tanh(s/2)), the
  normalizer cancels the extra factor), RMS rsqrt runs on the vector
  engine (quake-style bit trick + 2 Newton steps). Eliminates ~240
  ACT_TABLE_LOADs (~300us) the old sqrt/exp/gelu mix required.
- Head-split attention layout [128 = 64 streams x 2 head-groups]:
  produced directly by per-half QKV matmuls via tile_position, halving
  the per-partition free size of all attention vector ops.
- v-cache stored t-major ([128, T, 192]) so cache writes are contiguous.
- 1/rms folded into the q/k/v PSUM->SBUF copies (per-partition scale,
  duplicated to both head-group halves by a tiny identity matmul), so
  QKV matmuls run on the unnormalized residual concurrently with the
  rsqrt computation.
- finished-mask via gen[:,EOS] >= max(gen) instead of max_index.
"""

import math
import os

import ml_dtypes
import numpy as np

# ---- problem dims (hardcoded; kernel.py must be self-contained) ----
B, S, H = 2, 256, 1024
BH, NH, HD = 384, 8, 48
P_SEED = 4
S_C = 12
L = 4
V = 258
T = P_SEED + S_C            # 16 (cache capacity)
EOS = 257
SCALE = 1.0 / math.sqrt(HD)
NCORES = 8
NS = (B * S) // NCORES      # 64 streams per core
NIT = P_SEED + S_C - 1      # 15 position iterations (positions 0..14)
F32 = np.float32
BF16 = ml_dtypes.bfloat16
HBH = BH // 2               # 192: per-head-group feature count


def _pack_inputs(x, Wproj, attn_norm, Wq, Wk, Wv, Wo, ffn_norm, W1, W2, Wlm):
    """Host-side repack: fold norm gains into weight rows, cast to bf16,
    lay out k-tiles with the contraction dim on partitions."""
    x = np.asarray(x, F32).reshape(B * S, H)
    an = np.asarray(attn_norm, F32)      # [L, BH]
    fn = np.asarray(ffn_norm, F32)       # [L, BH]
    Wq, Wk, Wv = (np.asarray(w, F32) for w in (Wq, Wk, Wv))
    Wo, W1, W2 = (np.asarray(w, F32) for w in (Wo, W1, W2))
    Wproj = np.asarray(Wproj, F32)
    Wlm = np.asarray(Wlm, F32)

    # QKV concat, gain folded on input rows: [L, BH, 3*BH] -> [128, L, 3, 3, 384]
    wqkv = np.concatenate([an[:, :, None] * Wq, an[:, :, None] * Wk,
                           an[:, :, None] * Wv], axis=2)      # [L, BH, 1152]
    wqkv = wqkv.reshape(L, 3, 128, 3, BH).transpose(2, 0, 1, 3, 4)  # [128,L,3kt,3j,384]
    # Wo: [L, BH, BH] -> [128, L, 3, 384]
    wo = Wo.reshape(L, 3, 128, BH).transpose(2, 0, 1, 3)
    # W1 (gain folded), stored as lhsT tiles [128k, L, 3kt, 12mt, 128m]
    w1 = (fn[:, :, None] * W1).reshape(L, 3, 128, 12, 128).transpose(2, 0, 1, 3, 4)
    # W2: [L, 4BH, BH] -> [128, L, 12, 384]
    w2 = W2.reshape(L, 12, 128, BH).transpose(2, 0, 1, 3)
    # Wproj: [H, P*BH] -> [128, 8, 1536]
    wproj = Wproj.reshape(8, 128, P_SEED * BH).transpose(1, 0, 2)
    # Wlm: [BH, V] -> [128, 3, 258]
    wlm = Wlm.reshape(3, 128, V).transpose(1, 0, 2)

    ident = np.zeros((128, 64), F32)
    ident[:64] = np.eye(64, dtype=F32)
    ident[64:] = np.eye(64, dtype=F32)
    # [64, 128] = [I64 | I64]: duplicates a [64,1] column to both halves
    identdup = np.concatenate([np.eye(64, dtype=F32)] * 2, axis=1)

    shared = {
        "wqkv": np.ascontiguousarray(wqkv).astype(BF16),
        "wo": np.ascontiguousarray(wo).astype(BF16),
        "w1": np.ascontiguousarray(w1).astype(BF16),
        "w2": np.ascontiguousarray(w2).astype(BF16),
        "wproj": np.ascontiguousarray(wproj).astype(BF16),
        "wlm": np.ascontiguousarray(wlm).astype(BF16),
        "identf": ident,
        "identb": ident.astype(BF16),
        "identdup": identdup,
    }
    xs = [np.ascontiguousarray(x[c * NS:(c + 1) * NS]) for c in range(NCORES)]
    return shared, xs


def _build(nc, tc, ctx):
    import concourse.bass as bass
    import concourse.mybir as mybir

    dt = mybir.dt
    AF = mybir.ActivationFunctionType
    OP = mybir.AluOpType

    # ---- DRAM I/O ----
    d_x = nc.dram_tensor("xs", [NS, H], dt.float32, kind="ExternalInput").ap()
    d_wqkv = nc.dram_tensor("wqkv", [128, L, 3, 3, BH], dt.bfloat16, kind="ExternalInput").ap()
    d_wo = nc.dram_tensor("wo", [128, L, 3, BH], dt.bfloat16, kind="ExternalInput").ap()
    d_w1 = nc.dram_tensor("w1", [128, L, 3, 12, 128], dt.bfloat16, kind="ExternalInput").ap()
    d_w2 = nc.dram_tensor("w2", [128, L, 12, BH], dt.bfloat16, kind="ExternalInput").ap()
    d_wproj = nc.dram_tensor("wproj", [128, 8, P_SEED * BH], dt.bfloat16, kind="ExternalInput").ap()
    d_wlm = nc.dram_tensor("wlm", [128, 3, V], dt.bfloat16, kind="ExternalInput").ap()
    d_identf = nc.dram_tensor("identf", [128, 64], dt.float32, kind="ExternalInput").ap()
    d_identb = nc.dram_tensor("identb", [128, 64], dt.bfloat16, kind="ExternalInput").ap()
    d_identdup = nc.dram_tensor("identdup", [64, 128], dt.float32, kind="ExternalInput").ap()
    d_logits = nc.dram_tensor("logits", [NS, S_C, V], dt.float32, kind="ExternalOutput").ap()

    # ---- pools ----
    singles = ctx.enter_context(tc.tile_pool(name="singles", bufs=1))
    wpool = ctx.enter_context(tc.tile_pool(name="work", bufs=2))
    xpool = ctx.enter_context(tc.tile_pool(name="xi", bufs=6))
    spool = ctx.enter_context(tc.tile_pool(name="small", bufs=4))
    bigpool = ctx.enter_context(tc.tile_pool(name="big", bufs=2))
    pqk = ctx.enter_context(tc.tile_pool(name="pqk", bufs=1, space="PSUM"))
    pvp = ctx.enter_context(tc.tile_pool(name="pvp", bufs=1, space="PSUM"))
    ptr = ctx.enter_context(tc.tile_pool(name="ptr", bufs=1, space="PSUM"))
    prr = ctx.enter_context(tc.tile_pool(name="prr", bufs=1, space="PSUM"))
    pmm = ctx.enter_context(tc.tile_pool(name="pmm", bufs=2, space="PSUM"))
    pg = ctx.enter_context(tc.tile_pool(name="pg", bufs=2, space="PSUM"))

    # ---- persistent SBUF ----
    w_qkv = singles.tile([128, L, 3, 3, BH], dt.bfloat16)
    w_o = singles.tile([128, L, 3, BH], dt.bfloat16)
    w_1 = singles.tile([128, L, 3, 12, 128], dt.bfloat16)
    w_2 = singles.tile([128, L, 12, BH], dt.bfloat16)
    w_lm = singles.tile([128, 3, V], dt.bfloat16)
    identf = singles.tile([128, 64], dt.float32)
    identb = singles.tile([128, 64], dt.bfloat16)
    identdup = singles.tile([64, 128], dt.float32)
    # per-layer KV caches in head-split layout; partition p = 64*hg + s
    kcache = [singles.tile([128, T, HBH], dt.bfloat16, name=f"kc{i}", tag=f"kc{i}")
              for i in range(L)]
    vcache = [singles.tile([128, T, HBH], dt.bfloat16, name=f"vc{i}", tag=f"vc{i}")
              for i in range(L)]

    for dst, src in ((w_qkv, d_wqkv), (w_o, d_wo), (w_1, d_w1), (w_2, d_w2),
                     (w_lm, d_wlm), (identf, d_identf), (identb, d_identb),
                     (identdup, d_identdup)):
        nc.sync.dma_start(out=dst, in_=src)

    # consts / state
    czero = singles.tile([128, 1], dt.float32)
    magic_t = singles.tile([64, 1], dt.uint32)
    f_t = singles.tile([64, 1], dt.float32)   # finished (0/1)
    m_t = singles.tile([64, 1], dt.float32)   # 1 - finished
    nc.vector.memset(czero, 0.0)
    nc.vector.memset(magic_t, 0x5F3759DF)
    nc.vector.memset(f_t, 0.0)
    nc.vector.memset(m_t, 1.0)

    def rsqrt64(ms):
        """[64,1] f32 -> [64,1] f32 approx 1/sqrt(ms) on DVE (one Newton)."""
        sh = spool.tile([64, 1], dt.uint32, tag="sh")
        nc.vector.tensor_scalar(sh, ms.bitcast(dt.uint32), 1, None,
                                op0=OP.logical_shift_right)
        y = spool.tile([64, 1], dt.float32, tag="y")
        nc.vector.tensor_tensor(y.bitcast(dt.uint32), magic_t, sh, op=OP.subtract)
        t1 = spool.tile([64, 1], dt.float32, tag="t1")
        nc.vector.tensor_tensor(t1, y, y, op=OP.mult)
        nc.vector.tensor_tensor(t1, t1, ms, op=OP.mult)
        nc.vector.tensor_scalar(t1, t1, -0.5, 1.5, op0=OP.mult, op1=OP.add)
        r = spool.tile([64, 1], dt.float32, tag="r")
        nc.vector.tensor_tensor(r, y, t1, op=OP.mult)
        return r

    def rms_r(xi):
        """r = 1/sqrt(mean(xi^2)+eps) as [64,1] f32 (ACT square + DVE)."""
        sq = bigpool.tile([64, BH], dt.bfloat16, tag="sq", bufs=2)
        ssq = spool.tile([64, 1], dt.float32, tag="ssq")
        nc.scalar.activation(sq, xi, AF.Square, accum_out=ssq)
        ms = spool.tile([64, 1], dt.float32, tag="ms")
        nc.vector.tensor_scalar(ms, ssq, 1.0 / BH, 1e-5, op0=OP.mult, op1=OP.add)
        return rsqrt64(ms)

    def transpose_h(xt):
        """xt [64, 384] bf16 -> hT [128, 3, 64] bf16 (lhsT layout)."""
        tr = ptr.tile([128, 3, 64], dt.bfloat16, tag="tr")
        for c in range(3):
            nc.tensor.transpose(tr[:, c, :], xt[:, c * 128:(c + 1) * 128],
                                identb[0:64, :])
        hT = wpool.tile([128, 3, 64], dt.bfloat16, tag="hT")
        nc.vector.tensor_copy(hT, tr)
        return hT

    def block(l, it, xi, xt):
        """One transformer block at position `it`, layer l.
        xi: [64, BH] f32 residual; xt: [64, BH] bf16 cast of xi."""
        pos, nk = it, it + 1
        kc, vc = kcache[l], vcache[l]

        # --- rms scale r (ACT+DVE), runs concurrently with transpose+QKV ---
        r = rms_r(xi)
        rr_ps = prr.tile([128, 1], dt.float32, tag="rr")
        nc.tensor.matmul(rr_ps, lhsT=identdup, rhs=r, start=True, stop=True)
        rr = spool.tile([128, 1], dt.float32, tag="rr")
        nc.vector.tensor_copy(rr, rr_ps)

        # --- transpose + QKV on unnormalized xt (18 MMs, N=192) ---
        hT = transpose_h(xt)
        qk_ps = pqk.tile([128, 2, HBH], dt.float32, tag="qk")
        v_ps = pvp.tile([128, HBH], dt.float32, tag="vps")
        for hg in range(2):
            rows = slice(64 * hg, 64 * hg + 64)
            tp = (0, 64) if hg else None
            for j in range(3):
                dst = v_ps[rows, :] if j == 2 else qk_ps[rows, j, :]
                for c in range(3):
                    nc.tensor.matmul(dst, lhsT=hT[:, c, :],
                                     rhs=w_qkv[:, l, c, j, hg * HBH:(hg + 1) * HBH],
                                     start=(c == 0), stop=(c == 2), tile_position=tp)

        # --- q/k/v copies with r folded (per-partition scale) ---
        q_sb = wpool.tile([128, HBH], dt.bfloat16, tag="q")
        nc.vector.tensor_scalar_mul(q_sb, qk_ps[:, 0, :], rr)
        nc.scalar.activation(kc[:, pos, :], qk_ps[:, 1, :], AF.Copy, scale=rr)
        nc.scalar.activation(vc[:, pos, :], v_ps, AF.Copy, scale=rr)

        # --- scores ---
        prod = bigpool.tile([128, T, HBH], dt.bfloat16, tag="prod", bufs=1)
        # old keys only need q -> overlaps the k/v cache-write copies
        if nk > 1:
            nc.vector.tensor_mul(
                prod[:, :nk - 1, :], kc[:, :nk - 1, :],
                q_sb.unsqueeze(1).broadcast_to([128, nk - 1, HBH]))
        nc.vector.tensor_mul(
            prod[:, nk - 1:nk, :], kc[:, nk - 1:nk, :],
            q_sb.unsqueeze(1).broadcast_to([128, 1, HBH]))
        # tree-add the within-head reduction (bf16 2x mode; TENSOR_REDUCE
        # over 48 runs at 1x and is ~2x slower)
        pv4 = prod[:, :nk, :].rearrange("p t (h d) -> p t h d", d=HD)
        nc.vector.tensor_add(pv4[:, :, :, 0:24], pv4[:, :, :, 0:24],
                             pv4[:, :, :, 24:48])
        nc.vector.tensor_add(pv4[:, :, :, 0:12], pv4[:, :, :, 0:12],
                             pv4[:, :, :, 12:24])
        sc = wpool.tile([128, T, 4], dt.float32, tag="sc")
        nc.vector.reduce_sum(sc[:, :nk, :], pv4[:, :, :, 0:12],
                             axis=mybir.AxisListType.X)
        # --- softmax from tanh: e = (1+t)/(1-t) = exp(2*atanh(t)) = exp(s) ---
        t_t = wpool.tile([128, T, 4], dt.float32, tag="tt")
        nc.scalar.activation(t_t[:, :nk, :], sc[:, :nk, :], AF.Tanh,
                             scale=0.5 * SCALE)
        b_t = wpool.tile([128, T, 4], dt.float32, tag="bt")
        nc.vector.tensor_scalar(b_t[:, :nk, :], t_t[:, :nk, :], -1.0, 1.0,
                                op0=OP.mult, op1=OP.add)
        d_t = wpool.tile([128, T, 4], dt.float32, tag="dt")
        nc.vector.reciprocal(d_t[:, :nk, :], b_t[:, :nk, :])
        a_t = wpool.tile([128, T, 4], dt.float32, tag="at")
        nc.scalar.activation(a_t[:, :nk, :], t_t[:, :nk, :], AF.Copy,
                             bias=1.0, scale=1.0)
        e_t = wpool.tile([128, 4, T], dt.bfloat16, tag="e")
        nc.vector.tensor_mul(e_t[:, :, :nk].transpose([0, 2, 1]),
                             a_t[:, :nk, :], d_t[:, :nk, :])
        den = spool.tile([128, 4], dt.float32, tag="den")
        nc.vector.reduce_sum(den, e_t[:, :, :nk], axis=mybir.AxisListType.X)
        rden = spool.tile([128, 4], dt.float32, tag="rden")
        nc.vector.reciprocal(rden, den)
        p_sb = wpool.tile([128, 4, T], dt.bfloat16, tag="p")
        nc.vector.tensor_mul(p_sb[:, :, :nk], e_t[:, :, :nk],
                             rden.unsqueeze(2).broadcast_to([128, 4, nk]))

        # --- o = p . v (tree-add over keys; v is t-major) ---
        prod2 = bigpool.tile([128, T, HBH], dt.bfloat16, tag="prod", bufs=1)
        p_bc = (p_sb[:, :, :nk].transpose([0, 2, 1]).unsqueeze(3)
                .broadcast_to([128, nk, 4, HD]))
        nc.vector.tensor_mul(
            prod2[:, :nk, :].rearrange("p t (h d) -> p t h d", d=HD),
            vc[:, :nk, :].rearrange("p t (h d) -> p t h d", d=HD), p_bc)
        w = nk
        while w > 1:
            a = (w + 1) // 2
            rem = w - a
            nc.vector.tensor_add(prod2[:, 0:rem, :], prod2[:, 0:rem, :],
                                 prod2[:, a:w, :])
            w = a
        o_sb = prod2[:, 0, :]                       # [128, 192] bf16

        # --- oT: 6 transposes into lhsT layout [128, 3, 64] ---
        tro = ptr.tile([128, 3, 64], dt.bfloat16, tag="tr")
        for hg in range(2):
            for c in range(3):
                f = hg * HBH + c * 64
                kci, ro = divmod(f, 128)
                tp = (64 * hg, ro) if (hg or ro) else None
                nc.tensor.transpose(tro[ro:ro + 64, kci, :],
                                    o_sb[64 * hg:64 * hg + 64, c * 64:(c + 1) * 64],
                                    identb[64 * hg:64 * hg + 64, :], tile_position=tp)
        oT = wpool.tile([128, 3, 64], dt.bfloat16, tag="hT")
        nc.vector.tensor_copy(oT, tro)

        # --- out proj + residual ---
        o_ps = pmm.tile([64, BH], dt.float32, tag="mm")
        for c in range(3):
            nc.tensor.matmul(o_ps, lhsT=oT[:, c, :], rhs=w_o[:, l, c, :],
                             start=(c == 0), stop=(c == 2))
        x1 = xpool.tile([64, BH], dt.float32, tag="xi")
        nc.vector.tensor_add(x1, xi, o_ps)

        # --- FFN ---
        r2 = rms_r(x1)
        h2 = wpool.tile([64, BH], dt.bfloat16, tag="h")
        nc.vector.tensor_scalar_mul(h2, x1, r2)
        h2T = transpose_h(h2)
        g_sb = wpool.tile([128, 12, 64], dt.bfloat16, tag="g")
        for half in range(2):
            g_ps = pg.tile([128, 6, 64], dt.float32, tag="g")
            for mi in range(6):
                mt = half * 6 + mi
                for c in range(3):
                    nc.tensor.matmul(g_ps[:, mi, :], lhsT=w_1[:, l, c, mt, :],
                                     rhs=h2T[:, c, :],
                                     start=(c == 0), stop=(c == 2))
            nc.scalar.activation(g_sb[:, half * 6:(half + 1) * 6, :], g_ps,
                                 AF.Gelu, bias=czero, scale=1.0)
        f2_ps = pmm.tile([64, BH], dt.float32, tag="mm")
        for mt in range(12):
            nc.tensor.matmul(f2_ps, lhsT=g_sb[:, mt, :], rhs=w_2[:, l, mt, :],
                             start=(mt == 0), stop=(mt == 11))
        x2 = xpool.tile([64, BH], dt.float32, tag="xi")
        nc.vector.tensor_add(x2, x1, f2_ps)
        xt2 = xpool.tile([64, BH], dt.bfloat16, tag="xt")
        nc.vector.tensor_copy(xt2, x2)
        return x2, xt2

    # ---- projection of x into seed positions ----
    xs_sb = bigpool.tile([64, H], dt.float32, tag="x0", bufs=1)
    nc.sync.dma_start(out=xs_sb, in_=d_x)
    xsT = wpool.tile([128, 8, 64], dt.bfloat16, tag="xsT")
    for c in range(8):
        trx = ptr.tile([128, 64], dt.float32, tag="tr")
        nc.tensor.transpose(trx, xs_sb[:, c * 128:(c + 1) * 128],
                            identf[0:64, :])
        nc.vector.tensor_copy(xsT[:, c, :], trx)
    x0_sb = bigpool.tile([64, P_SEED * BH], dt.float32, tag="x0", bufs=1)
    for j in range(3):
        pp = pmm.tile([128, 512], dt.float32, tag="mm")
        for c in range(8):
            wpb = bigpool.tile([128, 512], dt.bfloat16, tag="wpb")
            nc.sync.dma_start(out=wpb, in_=d_wproj[:, c, j * 512:(j + 1) * 512])
            nc.tensor.matmul(pp[0:64, :], lhsT=xsT[:, c, :], rhs=wpb,
                             start=(c == 0), stop=(c == 7))
        nc.scalar.copy(x0_sb[:, j * 512:(j + 1) * 512], pp[0:64, :])

    # ---- prefill: 4x4 (position, layer) wavefront ----
    # block(it, l) depends only on block(it-1, l) [cache] and
    # block(it, l-1) [input]; seed inputs are independent, so
    # anti-diagonals are independent work the scheduler can overlap.
    x0v = x0_sb.rearrange("p (s d) -> p s d", d=BH)
    xis = {}
    xts = {}
    for it in range(P_SEED):
        xis[it] = x0v[:, it, :]
        t = xpool.tile([64, BH], dt.bfloat16, tag="xt")
        nc.vector.tensor_copy(t, xis[it])
        xts[it] = t
    for s in range(P_SEED + L - 1):
        for it in range(max(0, s - L + 1), min(P_SEED, s + 1)):
            l = s - it
            xis[it], xts[it] = block(l, it, xis[it], xts[it])
    xi, xt = xis[P_SEED - 1], xts[P_SEED - 1]

    # ---- decode loop ----
    for it in range(P_SEED - 1, NIT):
        if it > P_SEED - 1:
            for l in range(L):
                xi, xt = block(l, it, xi, xt)
        gen = xi
        step = it - (P_SEED - 1)
        # masked gen -> next input (+ logits source). m_t read BEFORE update.
        gm = xpool.tile([64, BH], dt.float32, tag="xi")
        nc.vector.tensor_scalar_mul(gm, gen, m_t)
        gmt = xpool.tile([64, BH], dt.bfloat16, tag="xt")
        nc.vector.tensor_copy(gmt, gm)
        # logits row
        gT = transpose_h(gmt)
        lm_ps = pmm.tile([64, V], dt.float32, tag="mm")
        for c in range(3):
            nc.tensor.matmul(lm_ps, lhsT=gT[:, c, :], rhs=w_lm[:, c, :],
                             start=(c == 0), stop=(c == 2))
        lm_sb = wpool.tile([64, V], dt.float32, tag="lm")
        nc.scalar.copy(lm_sb, lm_ps)
        nc.sync.dma_start(out=d_logits[:, step, :], in_=lm_sb)
        # finished update: argmax(gen)==EOS  <=>  gen[:,EOS] >= max(gen)
        mx8 = spool.tile([64, 8], dt.float32, tag="mx8")
        nc.vector.max(mx8, gen)
        mx = spool.tile([64, 1], dt.float32, tag="mx")
        nc.vector.reduce_max(mx, mx8, axis=mybir.AxisListType.X)
        eq = spool.tile([64, 1], dt.float32, tag="eq")
        nc.vector.tensor_tensor(eq, gen[:, EOS:EOS + 1], mx, op=OP.is_ge)
        eq2 = spool.tile([64, 1], dt.float32, tag="eq2")
        nc.vector.tensor_mul(eq2, eq, m_t)
        nc.vector.tensor_add(f_t, f_t, eq2)
        nc.vector.tensor_scalar(m_t, f_t, -1.0, 1.0, op0=OP.mult, op1=OP.add)
        xi, xt = gm, gmt


_CACHE = {}


def _get_compiled():
    if "nc" in _CACHE:
        return _CACHE["nc"]
    from contextlib import ExitStack

    import concourse.bacc as bacc
    import concourse.tile as tile

    nc = bacc.Bacc("TRN2", target_bir_lowering=False, debug=False,
                   num_devices=NCORES)
    with tile.TileContext(nc) as tc:
        with ExitStack() as ctx:
            _build(nc, tc, ctx)
    nc.compile()
    _CACHE["nc"] = nc
    return nc


def kernel(**inputs):
    from concourse.bass_utils import run_bass_kernel_spmd

    shared, xs = _pack_inputs(
        inputs["x"], inputs["Wproj"], inputs["attn_norm"], inputs["Wq"],
        inputs["Wk"], inputs["Wv"], inputs["Wo"], inputs["ffn_norm"],
        inputs["W1"], inputs["W2"], inputs["Wlm"])

    nc = _get_compiled()
    in_maps = [dict(shared, xs=xs[c]) for c in range(NCORES)]
    res = run_bass_kernel_spmd(nc, in_maps, core_ids=list(range(NCORES)),
                               trace=bool(int(os.environ.get("KERNEL_TRACE", "0"))))
    logits = np.concatenate([r["logits"] for r in res.results], axis=0)
    _CACHE["last_exec_ns"] = res.exec_time_ns
    return logits.reshape(B, S, S_C, V).astype(F32)


if __name__ == "__main__":
    nc = _get_compiled()
    print("built + compiled OK")


# revision 32
# speedup vs baseline: 1.1818x; 1.1818x over previous
"""Trainium2 Bass kernel for nn_ByteLevelDecoder.

Data-parallel over 512 byte streams, 64 per core on 8 cores. v2 design:
- Single ACT table set (gelu_and_others) for the whole kernel: softmax
  exp is computed from tanh (exp(s) = (1+tanh(s/2))/(1-tanh(s/2)), the
  normalizer cancels the extra factor), RMS rsqrt runs on the vector
  engine (quake-style bit trick + 2 Newton steps). Eliminates ~240
  ACT_TABLE_LOADs (~300us) the old sqrt/exp/gelu mix required.
- Head-split attention layout [128 = 64 streams x 2 head-groups]:
  produced directly by per-half QKV matmuls via tile_position, halving
  the per-partition free size of all attention vector ops.
- v-cache stored t-major ([128, T, 192]) so cache writes are contiguous.
- 1/rms folded into the q/k/v PSUM->SBUF copies (per-partition scale,
  duplicated to both head-group halves by a tiny identity matmul), so
  QKV matmuls run on the unnormalized residual concurrently with the
  rsqrt computation.
- finished-mask via gen[:,EOS] >= max(gen) instead of max_index.
"""

import math
import os

import ml_dtypes
import numpy as np

# ---- problem dims (hardcoded; kernel.py must be self-contained) ----
B, S, H = 2, 256, 1024
BH, NH, HD = 384, 8, 48
P_SEED = 4
S_C = 12
L = 4
V = 258
T = P_SEED + S_C            # 16 (cache capacity)
EOS = 257
SCALE = 1.0 / math.sqrt(HD)
NCORES = 8
NS = (B * S) // NCORES      # 64 streams per core
NIT = P_SEED + S_C - 1      # 15 position iterations (positions 0..14)
F32 = np.float32
BF16 = ml_dtypes.bfloat16
HBH = BH // 2               # 192: per-head-group feature count


def _pack_inputs(x, Wproj, attn_norm, Wq, Wk, Wv, Wo, ffn_norm, W1, W2, Wlm):
    """Host-side repack: fold norm gains into weight rows, cast to bf16,
    lay out k-tiles with the contraction dim on partitions."""
    x = np.asarray(x, F32).reshape(B * S, H)
    an = np.asarray(attn_norm, F32)      # [L, BH]
    fn = np.asarray(ffn_norm, F32)       # [L, BH]
    Wq, Wk, Wv = (np.asarray(w, F32) for w in (Wq, Wk, Wv))
    Wo, W1, W2 = (np.asarray(w, F32) for w in (Wo, W1, W2))
    Wproj = np.asarray(Wproj, F32)
    Wlm = np.asarray(Wlm, F32)

    # QKV concat, gain folded on input rows: [L, BH, 3*BH] -> [128, L, 3, 3, 384]
    wqkv = np.concatenate([an[:, :, None] * Wq, an[:, :, None] * Wk,
                           an[:, :, None] * Wv], axis=2)      # [L, BH, 1152]
    wqkv = wqkv.reshape(L, 3, 128, 3, BH).transpose(2, 0, 1, 3, 4)  # [128,L,3kt,3j,384]
    # Wo: [L, BH, BH] -> [128, L, 3, 384]
    wo = Wo.reshape(L, 3, 128, BH).transpose(2, 0, 1, 3)
    # W1 (gain folded), stored as lhsT tiles [128k, L, 3kt, 12mt, 128m]
    w1 = (fn[:, :, None] * W1).reshape(L, 3, 128, 12, 128).transpose(2, 0, 1, 3, 4)
    # W2: [L, 4BH, BH] -> [128, L, 12, 384]
    w2 = W2.reshape(L, 12, 128, BH).transpose(2, 0, 1, 3)
    # Wproj: [H, P*BH] -> [128, 8, 1536]
    wproj = Wproj.reshape(8, 128, P_SEED * BH).transpose(1, 0, 2)
    # Wlm: [BH, V] -> [128, 3, 258]
    wlm = Wlm.reshape(3, 128, V).transpose(1, 0, 2)

    ident = np.zeros((128, 64), F32)
    ident[:64] = np.eye(64, dtype=F32)
    ident[64:] = np.eye(64, dtype=F32)
    # [64, 128] = [I64 | I64]: duplicates a [64,1] column to both halves
    identdup = np.concatenate([np.eye(64, dtype=F32)] * 2, axis=1)

    shared = {
        "wqkv": np.ascontiguousarray(wqkv).astype(BF16),
        "wo": np.ascontiguousarray(wo).astype(BF16),
        "w1": np.ascontiguousarray(w1).astype(BF16),
        "w2": np.ascontiguousarray(w2).astype(BF16),
        "wproj": np.ascontiguousarray(wproj).astype(BF16),
        "wlm": np.ascontiguousarray(wlm).astype(BF16),
        "identf": ident,
        "identb": ident.astype(BF16),
        "identdup": identdup,
    }
    xs = [np.ascontiguousarray(x[c * NS:(c + 1) * NS]) for c in range(NCORES)]
    return shared, xs


def _build(nc, tc, ctx):
    import concourse.bass as bass
    import concourse.mybir as mybir

    dt = mybir.dt
    AF = mybir.ActivationFunctionType
    OP = mybir.AluOpType

    # ---- DRAM I/O ----
    d_x = nc.dram_tensor("xs", [NS, H], dt.float32, kind="ExternalInput").ap()
    d_wqkv = nc.dram_tensor("wqkv", [128, L, 3, 3, BH], dt.bfloat16, kind="ExternalInput").ap()
    d_wo = nc.dram_tensor("wo", [128, L, 3, BH], dt.bfloat16, kind="ExternalInput").ap()
    d_w1 = nc.dram_tensor("w1", [128, L, 3, 12, 128], dt.bfloat16, kind="ExternalInput").ap()
    d_w2 = nc.dram_tensor("w2", [128, L, 12, BH], dt.bfloat16, kind="ExternalInput").ap()
    d_wproj = nc.dram_tensor("wproj", [128, 8, P_SEED * BH], dt.bfloat16, kind="ExternalInput").ap()
    d_wlm = nc.dram_tensor("wlm", [128, 3, V], dt.bfloat16, kind="ExternalInput").ap()
    d_identf = nc.dram_tensor("identf", [128, 64], dt.float32, kind="ExternalInput").ap()
    d_identb = nc.dram_tensor("identb", [128, 64], dt.bfloat16, kind="ExternalInput").ap()
    d_identdup = nc.dram_tensor("identdup", [64, 128], dt.float32, kind="ExternalInput").ap()
    d_logits = nc.dram_tensor("logits", [NS, S_C, V], dt.float32, kind="ExternalOutput").ap()

    # ---- pools ----
    singles = ctx.enter_context(tc.tile_pool(name="singles", bufs=1))
    wpool = ctx.enter_context(tc.tile_pool(name="work", bufs=2))
    xpool = ctx.enter_context(tc.tile_pool(name="xi", bufs=6))
    spool = ctx.enter_context(tc.tile_pool(name="small", bufs=4))
    bigpool = ctx.enter_context(tc.tile_pool(name="big", bufs=2))
    pqk = ctx.enter_context(tc.tile_pool(name="pqk", bufs=1, space="PSUM"))
    pvp = ctx.enter_context(tc.tile_pool(name="pvp", bufs=1, space="PSUM"))
    ptr = ctx.enter_context(tc.tile_pool(name="ptr", bufs=1, space="PSUM"))
    prr = ctx.enter_context(tc.tile_pool(name="prr", bufs=1, space="PSUM"))
    pmm = ctx.enter_context(tc.tile_pool(name="pmm", bufs=2, space="PSUM"))
    pg = ctx.enter_context(tc.tile_pool(name="pg", bufs=2, space="PSUM"))

    # ---- persistent SBUF ----
    w_qkv = singles.tile([128, L, 3, 3, BH], dt.bfloat16)
    w_o = singles.tile([128, L, 3, BH], dt.bfloat16)
    w_1 = singles.tile([128, L, 3, 12, 128], dt.bfloat16)
    w_2 = singles.tile([128, L, 12, BH], dt.bfloat16)
    w_lm = singles.tile([128, 3, V], dt.bfloat16)
    identf = singles.tile([128, 64], dt.float32)
    identb = singles.tile([128, 64], dt.bfloat16)
    identdup = singles.tile([64, 128], dt.float32)
    # per-layer KV caches in head-split layout; partition p = 64*hg + s
    kcache = [singles.tile([128, T, HBH], dt.bfloat16, name=f"kc{i}", tag=f"kc{i}")
              for i in range(L)]
    vcache = [singles.tile([128, T, HBH], dt.bfloat16, name=f"vc{i}", tag=f"vc{i}")
              for i in range(L)]

    for dst, src in ((w_qkv, d_wqkv), (w_o, d_wo), (w_1, d_w1), (w_2, d_w2),
                     (w_lm, d_wlm), (identf, d_identf), (identb, d_identb),
                     (identdup, d_identdup)):
        nc.sync.dma_start(out=dst, in_=src)

    # consts / state
    czero = singles.tile([128, 1], dt.float32)
    magic_t = singles.tile([64, 1], dt.uint32)
    f_t = singles.tile([64, 1], dt.float32)   # finished (0/1)
    m_t = singles.tile([64, 1], dt.float32)   # 1 - finished
    nc.vector.memset(czero, 0.0)
    nc.vector.memset(magic_t, 0x5F3759DF)
    nc.vector.memset(f_t, 0.0)
    nc.vector.memset(m_t, 1.0)

    def rsqrt64(ms):
        """[64,1] f32 -> [64,1] f32 approx 1/sqrt(ms) on DVE (one Newton)."""
        sh = spool.tile([64, 1], dt.uint32, tag="sh")
        nc.vector.tensor_scalar(sh, ms.bitcast(dt.uint32), 1, None,
                                op0=OP.logical_shift_right)
        y = spool.tile([64, 1], dt.float32, tag="y")
        nc.vector.tensor_tensor(y.bitcast(dt.uint32), magic_t, sh, op=OP.subtract)
        t1 = spool.tile([64, 1], dt.float32, tag="t1")
        nc.vector.tensor_tensor(t1, y, y, op=OP.mult)
        nc.vector.tensor_tensor(t1, t1, ms, op=OP.mult)
        nc.vector.tensor_scalar(t1, t1, -0.5, 1.5, op0=OP.mult, op1=OP.add)
        r = spool.tile([64, 1], dt.float32, tag="r")
        nc.vector.tensor_tensor(r, y, t1, op=OP.mult)
        return r

    def rms_r(xi):
        """r = 1/sqrt(mean(xi^2)+eps) as [64,1] f32 (ACT square + DVE)."""
        sq = bigpool.tile([64, BH], dt.bfloat16, tag="sq", bufs=2)
        ssq = spool.tile([64, 1], dt.float32, tag="ssq")
        nc.scalar.activation(sq, xi, AF.Square, accum_out=ssq)
        ms = spool.tile([64, 1], dt.float32, tag="ms")
        nc.vector.tensor_scalar(ms, ssq, 1.0 / BH, 1e-5, op0=OP.mult, op1=OP.add)
        return rsqrt64(ms)

    def transpose_h(xt):
        """xt [64, 384] bf16 -> hT [128, 3, 64] bf16 (lhsT layout)."""
        tr = ptr.tile([128, 3, 64], dt.bfloat16, tag="tr")
        for c in range(3):
            nc.tensor.transpose(tr[:, c, :], xt[:, c * 128:(c + 1) * 128],
                                identb[0:64, :])
        hT = wpool.tile([128, 3, 64], dt.bfloat16, tag="hT")
        nc.vector.tensor_copy(hT, tr)
        return hT

    def block(l, it, xi, xt):
        """One transformer block at position `it`, layer l.
        xi: [64, BH] f32 residual; xt: [64, BH] bf16 cast of xi."""
        pos, nk = it, it + 1
        kc, vc = kcache[l], vcache[l]

        # --- rms scale r (ACT+DVE), runs concurrently with transpose+QKV ---
        r = rms_r(xi)
        rr_ps = prr.tile([128, 1], dt.float32, tag="rr")
        nc.tensor.matmul(rr_ps, lhsT=identdup, rhs=r, start=True, stop=True)
        rr = spool.tile([128, 1], dt.float32, tag="rr")
        nc.vector.tensor_copy(rr, rr_ps)

        # --- transpose + QKV on unnormalized xt (18 MMs, N=192) ---
        hT = transpose_h(xt)
        qk_ps = pqk.tile([128, 2, HBH], dt.float32, tag="qk")
        v_ps = pvp.tile([128, HBH], dt.float32, tag="vps")
        for hg in range(2):
            rows = slice(64 * hg, 64 * hg + 64)
            tp = (0, 64) if hg else None
            for j in range(3):
                dst = v_ps[rows, :] if j == 2 else qk_ps[rows, j, :]
                for c in range(3):
                    nc.tensor.matmul(dst, lhsT=hT[:, c, :],
                                     rhs=w_qkv[:, l, c, j, hg * HBH:(hg + 1) * HBH],
                                     start=(c == 0), stop=(c == 2), tile_position=tp)

        # --- q/k/v copies with r folded (per-partition scale) ---
        q_sb = wpool.tile([128, HBH], dt.bfloat16, tag="q")
        nc.vector.tensor_scalar_mul(q_sb, qk_ps[:, 0, :], rr)
        nc.scalar.activation(kc[:, pos, :], qk_ps[:, 1, :], AF.Copy, scale=rr)
        nc.scalar.activation(vc[:, pos, :], v_ps, AF.Copy, scale=rr)

        # --- scores ---
        prod = bigpool.tile([128, T, HBH], dt.bfloat16, tag="prod", bufs=1)
        # old keys only need q -> overlaps the k/v cache-write copies
        if nk > 1:
            nc.vector.tensor_mul(
                prod[:, :nk - 1, :], kc[:, :nk - 1, :],
                q_sb.unsqueeze(1).broadcast_to([128, nk - 1, HBH]))
        nc.vector.tensor_mul(
            prod[:, nk - 1:nk, :], kc[:, nk - 1:nk, :],
            q_sb.unsqueeze(1).broadcast_to([128, 1, HBH]))
        # tree-add the within-head reduction (bf16 2x mode; TENSOR_REDUCE
        # over 48 runs at 1x and is ~2x slower)
        pv4 = prod[:, :nk, :].rearrange("p t (h d) -> p t h d", d=HD)
        nc.vector.tensor_add(pv4[:, :, :, 0:24], pv4[:, :, :, 0:24],
                             pv4[:, :, :, 24:48])
        nc.vector.tensor_add(pv4[:, :, :, 0:12], pv4[:, :, :, 0:12],
                             pv4[:, :, :, 12:24])
        sc = wpool.tile([128, T, 4], dt.float32, tag="sc")
        nc.vector.reduce_sum(sc[:, :nk, :], pv4[:, :, :, 0:12],
                             axis=mybir.AxisListType.X)
        # --- softmax from tanh: e = (1+t)/(1-t) = exp(2*atanh(t)) = exp(s) ---
        t_t = wpool.tile([128, T, 4], dt.float32, tag="tt")
        nc.scalar.activation(t_t[:, :nk, :], sc[:, :nk, :], AF.Tanh,
                             scale=0.5 * SCALE)
        b_t = wpool.tile([128, T, 4], dt.float32, tag="bt")
        nc.scalar.activation(b_t[:, :nk, :], t_t[:, :nk, :], AF.Copy,
                             bias=1.0, scale=-1.0)
        d_t = wpool.tile([128, T, 4], dt.float32, tag="dt")
        nc.vector.reciprocal(d_t[:, :nk, :], b_t[:, :nk, :])
        a_t = wpool.tile([128, T, 4], dt.float32, tag="at")
        nc.scalar.activation(a_t[:, :nk, :], t_t[:, :nk, :], AF.Copy,
                             bias=1.0, scale=1.0)
        e_t = wpool.tile([128, 4, T], dt.bfloat16, tag="e")
        nc.vector.tensor_mul(e_t[:, :, :nk].transpose([0, 2, 1]),
                             a_t[:, :nk, :], d_t[:, :nk, :])
        den = spool.tile([128, 4], dt.float32, tag="den")
        nc.vector.reduce_sum(den, e_t[:, :, :nk], axis=mybir.AxisListType.X)
        rden = spool.tile([128, 4], dt.float32, tag="rden")
        nc.vector.reciprocal(rden, den)
        p_sb = wpool.tile([128, 4, T], dt.bfloat16, tag="p")
        nc.vector.tensor_mul(p_sb[:, :, :nk], e_t[:, :, :nk],
                             rden.unsqueeze(2).broadcast_to([128, 4, nk]))

        # --- o = p . v (tree-add over keys; v is t-major) ---
        prod2 = bigpool.tile([128, T, HBH], dt.bfloat16, tag="prod", bufs=1)
        p_bc = (p_sb[:, :, :nk].transpose([0, 2, 1]).unsqueeze(3)
                .broadcast_to([128, nk, 4, HD]))
        nc.vector.tensor_mul(
            prod2[:, :nk, :].rearrange("p t (h d) -> p t h d", d=HD),
            vc[:, :nk, :].rearrange("p t (h d) -> p t h d", d=HD), p_bc)
        w = nk
        while w > 1:
            a = (w + 1) // 2
            rem = w - a
            nc.vector.tensor_add(prod2[:, 0:rem, :], prod2[:, 0:rem, :],
                                 prod2[:, a:w, :])
            w = a
        o_sb = prod2[:, 0, :]                       # [128, 192] bf16

        # --- oT: 6 transposes into lhsT layout [128, 3, 64] ---
        tro = ptr.tile([128, 3, 64], dt.bfloat16, tag="tr")
        for hg in range(2):
            for c in range(3):
                f = hg * HBH + c * 64
                kci, ro = divmod(f, 128)
                tp = (64 * hg, ro) if (hg or ro) else None
                nc.tensor.transpose(tro[ro:ro + 64, kci, :],
                                    o_sb[64 * hg:64 * hg + 64, c * 64:(c + 1) * 64],
                                    identb[64 * hg:64 * hg + 64, :], tile_position=tp)
        oT = wpool.tile([128, 3, 64], dt.bfloat16, tag="hT")
        nc.scalar.copy(oT, tro)

        # --- out proj + residual ---
        o_ps = pmm.tile([64, BH], dt.float32, tag="mm")
        for c in range(3):
            nc.tensor.matmul(o_ps, lhsT=oT[:, c, :], rhs=w_o[:, l, c, :],
                             start=(c == 0), stop=(c == 2))
        x1 = xpool.tile([64, BH], dt.float32, tag="xi")
        nc.vector.tensor_add(x1, xi, o_ps)

        # --- FFN ---
        r2 = rms_r(x1)
        h2 = wpool.tile([64, BH], dt.bfloat16, tag="h")
        nc.vector.tensor_scalar_mul(h2, x1, r2)
        h2T = transpose_h(h2)
        g_sb = wpool.tile([128, 12, 64], dt.bfloat16, tag="g")
        for half in range(2):
            g_ps = pg.tile([128, 6, 64], dt.float32, tag="g")
            for mi in range(6):
                mt = half * 6 + mi
                for c in range(3):
                    nc.tensor.matmul(g_ps[:, mi, :], lhsT=w_1[:, l, c, mt, :],
                                     rhs=h2T[:, c, :],
                                     start=(c == 0), stop=(c == 2))
            nc.scalar.activation(g_sb[:, half * 6:(half + 1) * 6, :], g_ps,
                                 AF.Gelu, bias=czero, scale=1.0)
        f2_ps = pmm.tile([64, BH], dt.float32, tag="mm")
        for mt in range(12):
            nc.tensor.matmul(f2_ps, lhsT=g_sb[:, mt, :], rhs=w_2[:, l, mt, :],
                             start=(mt == 0), stop=(mt == 11))
        x2 = xpool.tile([64, BH], dt.float32, tag="xi")
        nc.vector.tensor_add(x2, x1, f2_ps)
        xt2 = xpool.tile([64, BH], dt.bfloat16, tag="xt")
        nc.vector.tensor_copy(xt2, x2)
        return x2, xt2

    # ---- projection of x into seed positions ----
    xs_sb = bigpool.tile([64, H], dt.float32, tag="x0", bufs=1)
    nc.sync.dma_start(out=xs_sb, in_=d_x)
    xsT = wpool.tile([128, 8, 64], dt.bfloat16, tag="xsT")
    for c in range(8):
        trx = ptr.tile([128, 64], dt.float32, tag="tr")
        nc.tensor.transpose(trx, xs_sb[:, c * 128:(c + 1) * 128],
                            identf[0:64, :])
        nc.vector.tensor_copy(xsT[:, c, :], trx)
    x0_sb = bigpool.tile([64, P_SEED * BH], dt.float32, tag="x0", bufs=1)
    for j in range(3):
        pp = pmm.tile([128, 512], dt.float32, tag="mm")
        for c in range(8):
            wpb = bigpool.tile([128, 512], dt.bfloat16, tag="wpb")
            nc.sync.dma_start(out=wpb, in_=d_wproj[:, c, j * 512:(j + 1) * 512])
            nc.tensor.matmul(pp[0:64, :], lhsT=xsT[:, c, :], rhs=wpb,
                             start=(c == 0), stop=(c == 7))
        nc.scalar.copy(x0_sb[:, j * 512:(j + 1) * 512], pp[0:64, :])

    # ---- prefill: 4x4 (position, layer) wavefront ----
    # block(it, l) depends only on block(it-1, l) [cache] and
    # block(it, l-1) [input]; seed inputs are independent, so
    # anti-diagonals are independent work the scheduler can overlap.
    x0v = x0_sb.rearrange("p (s d) -> p s d", d=BH)
    xis = {}
    xts = {}
    for it in range(P_SEED):
        xis[it] = x0v[:, it, :]
        t = xpool.tile([64, BH], dt.bfloat16, tag="xt")
        nc.vector.tensor_copy(t, xis[it])
        xts[it] = t
    for s in range(P_SEED + L - 1):
        for it in range(max(0, s - L + 1), min(P_SEED, s + 1)):
            l = s - it
            xis[it], xts[it] = block(l, it, xis[it], xts[it])
    xi, xt = xis[P_SEED - 1], xts[P_SEED - 1]

    # ---- decode loop ----
    for it in range(P_SEED - 1, NIT):
        if it > P_SEED - 1:
            for l in range(L):
                xi, xt = block(l, it, xi, xt)
        gen = xi
        step = it - (P_SEED - 1)
        # masked gen -> next input (+ logits source). m_t read BEFORE update.
        gm = xpool.tile([64, BH], dt.float32, tag="xi")
        nc.vector.tensor_scalar_mul(gm, gen, m_t)
        gmt = xpool.tile([64, BH], dt.bfloat16, tag="xt")
        nc.vector.tensor_copy(gmt, gm)
        # logits row
        gT = transpose_h(gmt)
        lm_ps = pmm.tile([64, V], dt.float32, tag="mm")
        for c in range(3):
            nc.tensor.matmul(lm_ps, lhsT=gT[:, c, :], rhs=w_lm[:, c, :],
                             start=(c == 0), stop=(c == 2))
        lm_sb = wpool.tile([64, V], dt.float32, tag="lm")
        nc.scalar.copy(lm_sb, lm_ps)
        nc.sync.dma_start(out=d_logits[:, step, :], in_=lm_sb)
        # finished update: argmax(gen)==EOS  <=>  gen[:,EOS] >= max(gen)
        mx8 = spool.tile([64, 8], dt.float32, tag="mx8")
        nc.vector.max(mx8, gen)
        mx = spool.tile([64, 1], dt.float32, tag="mx")
        nc.vector.reduce_max(mx, mx8, axis=mybir.AxisListType.X)
        eq = spool.tile([64, 1], dt.float32, tag="eq")
        nc.vector.tensor_tensor(eq, gen[:, EOS:EOS + 1], mx, op=OP.is_ge)
        eq2 = spool.tile([64, 1], dt.float32, tag="eq2")
        nc.vector.tensor_mul(eq2, eq, m_t)
        nc.vector.tensor_add(f_t, f_t, eq2)
        nc.vector.tensor_scalar(m_t, f_t, -1.0, 1.0, op0=OP.mult, op1=OP.add)
        xi, xt = gm, gmt


_CACHE = {}


def _get_compiled():
    if "nc" in _CACHE:
        return _CACHE["nc"]
    from contextlib import ExitStack

    import concourse.bacc as bacc
    import concourse.tile as tile

    nc = bacc.Bacc("TRN2", target_bir_lowering=False, debug=False,
                   num_devices=NCORES)
    with tile.TileContext(nc) as tc:
        with ExitStack() as ctx:
            _build(nc, tc, ctx)
    nc.compile()
    _CACHE["nc"] = nc
    return nc


def kernel(**inputs):
    from concourse.bass_utils import run_bass_kernel_spmd

    shared, xs = _pack_inputs(
        inputs["x"], inputs["Wproj"], inputs["attn_norm"], inputs["Wq"],
        inputs["Wk"], inputs["Wv"], inputs["Wo"], inputs["ffn_norm"],
        inputs["W1"], inputs["W2"], inputs["Wlm"])

    nc = _get_compiled()
    in_maps = [dict(shared, xs=xs[c]) for c in range(NCORES)]
    res = run_bass_kernel_spmd(nc, in_maps, core_ids=list(range(NCORES)),
                               trace=bool(int(os.environ.get("KERNEL_TRACE", "0"))))
    logits = np.concatenate([r["logits"] for r in res.results], axis=0)
    _CACHE["last_exec_ns"] = res.exec_time_ns
    return logits.reshape(B, S, S_C, V).astype(F32)


if __name__ == "__main__":
    nc = _get_compiled()
    print("built + compiled OK")


# revision 34
# speedup vs baseline: 1.1874x; 1.0047x over previous
"""Trainium2 Bass kernel for nn_ByteLevelDecoder.

Data-parallel over 512 byte streams, 64 per core on 8 cores. v2 design:
- Single ACT table set (gelu_and_others) for the whole kernel: softmax
  exp is computed from tanh (exp(s) = (1+tanh(s/2))/(1-tanh(s/2)), the
  normalizer cancels the extra factor), RMS rsqrt runs on the vector
  engine (quake-style bit trick + 2 Newton steps). Eliminates ~240
  ACT_TABLE_LOADs (~300us) the old sqrt/exp/gelu mix required.
- Head-split attention layout [128 = 64 streams x 2 head-groups]:
  produced directly by per-half QKV matmuls via tile_position, halving
  the per-partition free size of all attention vector ops.
- v-cache stored t-major ([128, T, 192]) so cache writes are contiguous.
- 1/rms folded into the q/k/v PSUM->SBUF copies (per-partition scale,
  duplicated to both head-group halves by a tiny identity matmul), so
  QKV matmuls run on the unnormalized residual concurrently with the
  rsqrt computation.
- finished-mask via gen[:,EOS] >= max(gen) instead of max_index.
"""

import math
import os

import ml_dtypes
import numpy as np

# ---- problem dims (hardcoded; kernel.py must be self-contained) ----
B, S, H = 2, 256, 1024
BH, NH, HD = 384, 8, 48
P_SEED = 4
S_C = 12
L = 4
V = 258
T = P_SEED + S_C            # 16 (cache capacity)
EOS = 257
SCALE = 1.0 / math.sqrt(HD)
NCORES = 8
NS = (B * S) // NCORES      # 64 streams per core
NIT = P_SEED + S_C - 1      # 15 position iterations (positions 0..14)
F32 = np.float32
BF16 = ml_dtypes.bfloat16
HBH = BH // 2               # 192: per-head-group feature count


def _pack_inputs(x, Wproj, attn_norm, Wq, Wk, Wv, Wo, ffn_norm, W1, W2, Wlm):
    """Host-side repack: fold norm gains into weight rows, cast to bf16,
    lay out k-tiles with the contraction dim on partitions."""
    x = np.asarray(x, F32).reshape(B * S, H)
    an = np.asarray(attn_norm, F32)      # [L, BH]
    fn = np.asarray(ffn_norm, F32)       # [L, BH]
    Wq, Wk, Wv = (np.asarray(w, F32) for w in (Wq, Wk, Wv))
    Wo, W1, W2 = (np.asarray(w, F32) for w in (Wo, W1, W2))
    Wproj = np.asarray(Wproj, F32)
    Wlm = np.asarray(Wlm, F32)

    # QKV concat, gain folded on input rows: [L, BH, 3*BH] -> [128, L, 3, 3, 384]
    wqkv = np.concatenate([an[:, :, None] * Wq, an[:, :, None] * Wk,
                           an[:, :, None] * Wv], axis=2)      # [L, BH, 1152]
    wqkv = wqkv.reshape(L, 3, 128, 3, BH).transpose(2, 0, 1, 3, 4)  # [128,L,3kt,3j,384]
    # Wo: [L, BH, BH] -> [128, L, 3, 384]
    wo = Wo.reshape(L, 3, 128, BH).transpose(2, 0, 1, 3)
    # W1 (gain folded), stored as lhsT tiles [128k, L, 3kt, 12mt, 128m]
    w1 = (fn[:, :, None] * W1).reshape(L, 3, 128, 12, 128).transpose(2, 0, 1, 3, 4)
    # W2: [L, 4BH, BH] -> [128, L, 12, 384]
    w2 = W2.reshape(L, 12, 128, BH).transpose(2, 0, 1, 3)
    # Wproj: [H, P*BH] -> [128, 8, 1536]
    wproj = Wproj.reshape(8, 128, P_SEED * BH).transpose(1, 0, 2)
    # Wlm: [BH, V] -> [128, 3, 258]
    wlm = Wlm.reshape(3, 128, V).transpose(1, 0, 2)

    ident = np.zeros((128, 64), F32)
    ident[:64] = np.eye(64, dtype=F32)
    ident[64:] = np.eye(64, dtype=F32)
    # [64, 128] = [I64 | I64]: duplicates a [64,1] column to both halves
    identdup = np.concatenate([np.eye(64, dtype=F32)] * 2, axis=1)

    shared = {
        "wqkv": np.ascontiguousarray(wqkv).astype(BF16),
        "wo": np.ascontiguousarray(wo).astype(BF16),
        "w1": np.ascontiguousarray(w1).astype(BF16),
        "w2": np.ascontiguousarray(w2).astype(BF16),
        "wproj": np.ascontiguousarray(wproj).astype(BF16),
        "wlm": np.ascontiguousarray(wlm).astype(BF16),
        "identf": ident,
        "identb": ident.astype(BF16),
        "identdup": identdup,
    }
    xs = [np.ascontiguousarray(x[c * NS:(c + 1) * NS]) for c in range(NCORES)]
    return shared, xs


def _build(nc, tc, ctx):
    import concourse.bass as bass
    import concourse.mybir as mybir

    dt = mybir.dt
    AF = mybir.ActivationFunctionType
    OP = mybir.AluOpType

    # ---- DRAM I/O ----
    d_x = nc.dram_tensor("xs", [NS, H], dt.float32, kind="ExternalInput").ap()
    d_wqkv = nc.dram_tensor("wqkv", [128, L, 3, 3, BH], dt.bfloat16, kind="ExternalInput").ap()
    d_wo = nc.dram_tensor("wo", [128, L, 3, BH], dt.bfloat16, kind="ExternalInput").ap()
    d_w1 = nc.dram_tensor("w1", [128, L, 3, 12, 128], dt.bfloat16, kind="ExternalInput").ap()
    d_w2 = nc.dram_tensor("w2", [128, L, 12, BH], dt.bfloat16, kind="ExternalInput").ap()
    d_wproj = nc.dram_tensor("wproj", [128, 8, P_SEED * BH], dt.bfloat16, kind="ExternalInput").ap()
    d_wlm = nc.dram_tensor("wlm", [128, 3, V], dt.bfloat16, kind="ExternalInput").ap()
    d_identf = nc.dram_tensor("identf", [128, 64], dt.float32, kind="ExternalInput").ap()
    d_identb = nc.dram_tensor("identb", [128, 64], dt.bfloat16, kind="ExternalInput").ap()
    d_identdup = nc.dram_tensor("identdup", [64, 128], dt.float32, kind="ExternalInput").ap()
    d_logits = nc.dram_tensor("logits", [NS, S_C, V], dt.float32, kind="ExternalOutput").ap()

    # ---- pools ----
    singles = ctx.enter_context(tc.tile_pool(name="singles", bufs=1))
    wpool = ctx.enter_context(tc.tile_pool(name="work", bufs=2))
    xpool = ctx.enter_context(tc.tile_pool(name="xi", bufs=6))
    spool = ctx.enter_context(tc.tile_pool(name="small", bufs=4))
    bigpool = ctx.enter_context(tc.tile_pool(name="big", bufs=2))
    pqk = ctx.enter_context(tc.tile_pool(name="pqk", bufs=1, space="PSUM"))
    pvp = ctx.enter_context(tc.tile_pool(name="pvp", bufs=1, space="PSUM"))
    ptr = ctx.enter_context(tc.tile_pool(name="ptr", bufs=1, space="PSUM"))
    prr = ctx.enter_context(tc.tile_pool(name="prr", bufs=1, space="PSUM"))
    pmm = ctx.enter_context(tc.tile_pool(name="pmm", bufs=2, space="PSUM"))
    pg = ctx.enter_context(tc.tile_pool(name="pg", bufs=2, space="PSUM"))

    # ---- persistent SBUF ----
    w_qkv = singles.tile([128, L, 3, 3, BH], dt.bfloat16)
    w_o = singles.tile([128, L, 3, BH], dt.bfloat16)
    w_1 = singles.tile([128, L, 3, 12, 128], dt.bfloat16)
    w_2 = singles.tile([128, L, 12, BH], dt.bfloat16)
    w_lm = singles.tile([128, 3, V], dt.bfloat16)
    identf = singles.tile([128, 64], dt.float32)
    identb = singles.tile([128, 64], dt.bfloat16)
    identdup = singles.tile([64, 128], dt.float32)
    # per-layer KV caches in head-split layout; partition p = 64*hg + s
    kcache = [singles.tile([128, T, HBH], dt.bfloat16, name=f"kc{i}", tag=f"kc{i}")
              for i in range(L)]
    vcache = [singles.tile([128, T, HBH], dt.bfloat16, name=f"vc{i}", tag=f"vc{i}")
              for i in range(L)]

    for dst, src in ((w_qkv, d_wqkv), (w_o, d_wo), (w_1, d_w1), (w_2, d_w2),
                     (w_lm, d_wlm), (identf, d_identf), (identb, d_identb),
                     (identdup, d_identdup)):
        nc.sync.dma_start(out=dst, in_=src)

    # consts / state
    czero = singles.tile([128, 1], dt.float32)
    magic_t = singles.tile([64, 1], dt.uint32)
    f_t = singles.tile([64, 1], dt.float32)   # finished (0/1)
    m_t = singles.tile([64, 1], dt.float32)   # 1 - finished
    nc.vector.memset(czero, 0.0)
    nc.vector.memset(magic_t, 0x5F3759DF)
    nc.vector.memset(f_t, 0.0)
    nc.vector.memset(m_t, 1.0)

    def rsqrt64(ms):
        """[64,1] f32 -> [64,1] f32 approx 1/sqrt(ms) on DVE (one Newton)."""
        sh = spool.tile([64, 1], dt.uint32, tag="sh")
        nc.vector.tensor_scalar(sh, ms.bitcast(dt.uint32), 1, None,
                                op0=OP.logical_shift_right)
        y = spool.tile([64, 1], dt.float32, tag="y")
        nc.vector.tensor_tensor(y.bitcast(dt.uint32), magic_t, sh, op=OP.subtract)
        t1 = spool.tile([64, 1], dt.float32, tag="t1")
        nc.vector.tensor_tensor(t1, y, y, op=OP.mult)
        nc.vector.tensor_tensor(t1, t1, ms, op=OP.mult)
        nc.vector.tensor_scalar(t1, t1, -0.5, 1.5, op0=OP.mult, op1=OP.add)
        r = spool.tile([64, 1], dt.float32, tag="r")
        nc.vector.tensor_tensor(r, y, t1, op=OP.mult)
        return r

    def rms_r(xi):
        """r = 1/sqrt(mean(xi^2)+eps) as [64,1] f32 (ACT square + DVE)."""
        sq = bigpool.tile([64, BH], dt.bfloat16, tag="sq", bufs=2)
        ssq = spool.tile([64, 1], dt.float32, tag="ssq")
        nc.scalar.activation(sq, xi, AF.Square, accum_out=ssq)
        ms = spool.tile([64, 1], dt.float32, tag="ms")
        nc.vector.tensor_scalar(ms, ssq, 1.0 / BH, 1e-5, op0=OP.mult, op1=OP.add)
        return rsqrt64(ms)

    def transpose_h(xt):
        """xt [64, 384] bf16 -> hT [128, 3, 64] bf16 (lhsT layout)."""
        tr = ptr.tile([128, 3, 64], dt.bfloat16, tag="tr")
        for c in range(3):
            nc.tensor.transpose(tr[:, c, :], xt[:, c * 128:(c + 1) * 128],
                                identb[0:64, :])
        hT = wpool.tile([128, 3, 64], dt.bfloat16, tag="hT")
        nc.vector.tensor_copy(hT, tr)
        return hT

    def block(l, it, xi, xt):
        """One transformer block at position `it`, layer l.
        xi: [64, BH] f32 residual; xt: [64, BH] bf16 cast of xi."""
        pos, nk = it, it + 1
        kc, vc = kcache[l], vcache[l]

        # --- rms scale r (ACT+DVE), runs concurrently with transpose+QKV ---
        r = rms_r(xi)
        rr_ps = prr.tile([128, 1], dt.float32, tag="rr")
        nc.tensor.matmul(rr_ps, lhsT=identdup, rhs=r, start=True, stop=True)
        rr = spool.tile([128, 1], dt.float32, tag="rr")
        nc.vector.tensor_copy(rr, rr_ps)

        # --- transpose + QKV on unnormalized xt (18 MMs, N=192) ---
        hT = transpose_h(xt)
        qk_ps = pqk.tile([128, 2, HBH], dt.float32, tag="qk")
        v_ps = pvp.tile([128, HBH], dt.float32, tag="vps")
        for hg in range(2):
            rows = slice(64 * hg, 64 * hg + 64)
            tp = (0, 64) if hg else None
            for j in range(3):
                dst = v_ps[rows, :] if j == 2 else qk_ps[rows, j, :]
                for c in range(3):
                    nc.tensor.matmul(dst, lhsT=hT[:, c, :],
                                     rhs=w_qkv[:, l, c, j, hg * HBH:(hg + 1) * HBH],
                                     start=(c == 0), stop=(c == 2), tile_position=tp)

        # --- q/k/v copies with r folded (per-partition scale) ---
        q_sb = wpool.tile([128, HBH], dt.bfloat16, tag="q")
        nc.vector.tensor_scalar_mul(q_sb, qk_ps[:, 0, :], rr)
        nc.scalar.activation(kc[:, pos, :], qk_ps[:, 1, :], AF.Copy, scale=rr)
        nc.scalar.activation(vc[:, pos, :], v_ps, AF.Copy, scale=rr)

        # --- scores ---
        prod = bigpool.tile([128, T, HBH], dt.bfloat16, tag="prod", bufs=1)
        # old keys only need q -> overlaps the k/v cache-write copies
        if nk > 1:
            nc.vector.tensor_mul(
                prod[:, :nk - 1, :], kc[:, :nk - 1, :],
                q_sb.unsqueeze(1).broadcast_to([128, nk - 1, HBH]))
        nc.vector.tensor_mul(
            prod[:, nk - 1:nk, :], kc[:, nk - 1:nk, :],
            q_sb.unsqueeze(1).broadcast_to([128, 1, HBH]))
        # tree-add the within-head reduction (bf16 2x mode; TENSOR_REDUCE
        # over 48 runs at 1x and is ~2x slower)
        pv4 = prod[:, :nk, :].rearrange("p t (h d) -> p t h d", d=HD)
        nc.vector.tensor_add(pv4[:, :, :, 0:24], pv4[:, :, :, 0:24],
                             pv4[:, :, :, 24:48])
        nc.vector.tensor_add(pv4[:, :, :, 0:12], pv4[:, :, :, 0:12],
                             pv4[:, :, :, 12:24])
        sc = wpool.tile([128, T, 4], dt.float32, tag="sc")
        nc.vector.reduce_sum(sc[:, :nk, :], pv4[:, :, :, 0:12],
                             axis=mybir.AxisListType.X)
        # --- softmax from tanh: e = (1+t)/(1-t) = exp(2*atanh(t)) = exp(s) ---
        t_t = wpool.tile([128, T, 4], dt.float32, tag="tt")
        nc.scalar.activation(t_t[:, :nk, :], sc[:, :nk, :], AF.Tanh,
                             scale=0.5 * SCALE)
        b_t = wpool.tile([128, T, 4], dt.float32, tag="bt")
        nc.scalar.activation(b_t[:, :nk, :], t_t[:, :nk, :], AF.Copy,
                             bias=1.0, scale=-1.0)
        d_t = wpool.tile([128, T, 4], dt.float32, tag="dt")
        nc.vector.reciprocal(d_t[:, :nk, :], b_t[:, :nk, :])
        a_t = wpool.tile([128, T, 4], dt.float32, tag="at")
        nc.scalar.activation(a_t[:, :nk, :], t_t[:, :nk, :], AF.Copy,
                             bias=1.0, scale=1.0)
        e_t = wpool.tile([128, 4, T], dt.bfloat16, tag="e")
        nc.vector.tensor_mul(e_t[:, :, :nk].transpose([0, 2, 1]),
                             a_t[:, :nk, :], d_t[:, :nk, :])
        den = spool.tile([128, 4], dt.float32, tag="den")
        nc.vector.reduce_sum(den, e_t[:, :, :nk], axis=mybir.AxisListType.X)
        rden = spool.tile([128, 4], dt.float32, tag="rden")
        nc.vector.reciprocal(rden, den)

        # --- o = (e . v) / den: normalization deferred to o so the
        # denominator reciprocal overlaps the e.v product ---
        prod2 = bigpool.tile([128, T, HBH], dt.bfloat16, tag="prod", bufs=1)
        p_bc = (e_t[:, :, :nk].transpose([0, 2, 1]).unsqueeze(3)
                .broadcast_to([128, nk, 4, HD]))
        nc.vector.tensor_mul(
            prod2[:, :nk, :].rearrange("p t (h d) -> p t h d", d=HD),
            vc[:, :nk, :].rearrange("p t (h d) -> p t h d", d=HD), p_bc)
        w = nk
        while w > 1:
            a = (w + 1) // 2
            rem = w - a
            nc.vector.tensor_add(prod2[:, 0:rem, :], prod2[:, 0:rem, :],
                                 prod2[:, a:w, :])
            w = a
        o_sb = prod2[:, 0, :]                       # [128, 192] bf16
        nc.vector.tensor_mul(
            o_sb.rearrange("p (h d) -> p h d", d=HD),
            o_sb.rearrange("p (h d) -> p h d", d=HD),
            rden.unsqueeze(2).broadcast_to([128, 4, HD]))

        # --- oT: 6 transposes into lhsT layout [128, 3, 64] ---
        tro = ptr.tile([128, 3, 64], dt.bfloat16, tag="tr")
        for hg in range(2):
            for c in range(3):
                f = hg * HBH + c * 64
                kci, ro = divmod(f, 128)
                tp = (64 * hg, ro) if (hg or ro) else None
                nc.tensor.transpose(tro[ro:ro + 64, kci, :],
                                    o_sb[64 * hg:64 * hg + 64, c * 64:(c + 1) * 64],
                                    identb[64 * hg:64 * hg + 64, :], tile_position=tp)
        oT = wpool.tile([128, 3, 64], dt.bfloat16, tag="hT")
        nc.scalar.copy(oT, tro)

        # --- out proj + residual ---
        o_ps = pmm.tile([64, BH], dt.float32, tag="mm")
        for c in range(3):
            nc.tensor.matmul(o_ps, lhsT=oT[:, c, :], rhs=w_o[:, l, c, :],
                             start=(c == 0), stop=(c == 2))
        x1 = xpool.tile([64, BH], dt.float32, tag="xi")
        nc.vector.tensor_add(x1, xi, o_ps)

        # --- FFN ---
        r2 = rms_r(x1)
        h2 = wpool.tile([64, BH], dt.bfloat16, tag="h")
        nc.vector.tensor_scalar_mul(h2, x1, r2)
        h2T = transpose_h(h2)
        g_sb = wpool.tile([128, 12, 64], dt.bfloat16, tag="g")
        for half in range(2):
            g_ps = pg.tile([128, 6, 64], dt.float32, tag="g")
            for mi in range(6):
                mt = half * 6 + mi
                for c in range(3):
                    nc.tensor.matmul(g_ps[:, mi, :], lhsT=w_1[:, l, c, mt, :],
                                     rhs=h2T[:, c, :],
                                     start=(c == 0), stop=(c == 2))
            nc.scalar.activation(g_sb[:, half * 6:(half + 1) * 6, :], g_ps,
                                 AF.Gelu, bias=czero, scale=1.0)
        f2_ps = pmm.tile([64, BH], dt.float32, tag="mm")
        for mt in range(12):
            nc.tensor.matmul(f2_ps, lhsT=g_sb[:, mt, :], rhs=w_2[:, l, mt, :],
                             start=(mt == 0), stop=(mt == 11))
        x2 = xpool.tile([64, BH], dt.float32, tag="xi")
        nc.vector.tensor_add(x2, x1, f2_ps)
        xt2 = xpool.tile([64, BH], dt.bfloat16, tag="xt")
        nc.vector.tensor_copy(xt2, x2)
        return x2, xt2

    # ---- projection of x into seed positions ----
    xs_sb = bigpool.tile([64, H], dt.float32, tag="x0", bufs=1)
    nc.sync.dma_start(out=xs_sb, in_=d_x)
    xsT = wpool.tile([128, 8, 64], dt.bfloat16, tag="xsT")
    for c in range(8):
        trx = ptr.tile([128, 64], dt.float32, tag="tr")
        nc.tensor.transpose(trx, xs_sb[:, c * 128:(c + 1) * 128],
                            identf[0:64, :])
        nc.vector.tensor_copy(xsT[:, c, :], trx)
    x0_sb = bigpool.tile([64, P_SEED * BH], dt.float32, tag="x0", bufs=1)
    for j in range(3):
        pp = pmm.tile([128, 512], dt.float32, tag="mm")
        for c in range(8):
            wpb = bigpool.tile([128, 512], dt.bfloat16, tag="wpb")
            nc.sync.dma_start(out=wpb, in_=d_wproj[:, c, j * 512:(j + 1) * 512])
            nc.tensor.matmul(pp[0:64, :], lhsT=xsT[:, c, :], rhs=wpb,
                             start=(c == 0), stop=(c == 7))
        nc.scalar.copy(x0_sb[:, j * 512:(j + 1) * 512], pp[0:64, :])

    # ---- prefill: 4x4 (position, layer) wavefront ----
    # block(it, l) depends only on block(it-1, l) [cache] and
    # block(it, l-1) [input]; seed inputs are independent, so
    # anti-diagonals are independent work the scheduler can overlap.
    x0v = x0_sb.rearrange("p (s d) -> p s d", d=BH)
    xis = {}
    xts = {}
    for it in range(P_SEED):
        xis[it] = x0v[:, it, :]
        t = xpool.tile([64, BH], dt.bfloat16, tag="xt")
        nc.vector.tensor_copy(t, xis[it])
        xts[it] = t
    for s in range(P_SEED + L - 1):
        for it in range(max(0, s - L + 1), min(P_SEED, s + 1)):
            l = s - it
            xis[it], xts[it] = block(l, it, xis[it], xts[it])
    xi, xt = xis[P_SEED - 1], xts[P_SEED - 1]

    # ---- decode loop ----
    for it in range(P_SEED - 1, NIT):
        if it > P_SEED - 1:
            for l in range(L):
                xi, xt = block(l, it, xi, xt)
        gen = xi
        step = it - (P_SEED - 1)
        # masked gen -> next input (+ logits source). m_t read BEFORE update.
        gm = xpool.tile([64, BH], dt.float32, tag="xi")
        nc.vector.tensor_scalar_mul(gm, gen, m_t)
        gmt = xpool.tile([64, BH], dt.bfloat16, tag="xt")
        nc.vector.tensor_copy(gmt, gm)
        # logits row
        gT = transpose_h(gmt)
        lm_ps = pmm.tile([64, V], dt.float32, tag="mm")
        for c in range(3):
            nc.tensor.matmul(lm_ps, lhsT=gT[:, c, :], rhs=w_lm[:, c, :],
                             start=(c == 0), stop=(c == 2))
        lm_sb = wpool.tile([64, V], dt.float32, tag="lm")
        nc.scalar.copy(lm_sb, lm_ps)
        nc.sync.dma_start(out=d_logits[:, step, :], in_=lm_sb)
        # finished update: argmax(gen)==EOS  <=>  gen[:,EOS] >= max(gen)
        mx8 = spool.tile([64, 8], dt.float32, tag="mx8")
        nc.vector.max(mx8, gen)
        mx = spool.tile([64, 1], dt.float32, tag="mx")
        nc.vector.reduce_max(mx, mx8, axis=mybir.AxisListType.X)
        eq = spool.tile([64, 1], dt.float32, tag="eq")
        nc.vector.tensor_tensor(eq, gen[:, EOS:EOS + 1], mx, op=OP.is_ge)
        eq2 = spool.tile([64, 1], dt.float32, tag="eq2")
        nc.vector.tensor_mul(eq2, eq, m_t)
        nc.vector.tensor_add(f_t, f_t, eq2)
        nc.vector.tensor_scalar(m_t, f_t, -1.0, 1.0, op0=OP.mult, op1=OP.add)
        xi, xt = gm, gmt


_CACHE = {}


def _get_compiled():
    if "nc" in _CACHE:
        return _CACHE["nc"]
    from contextlib import ExitStack

    import concourse.bacc as bacc
    import concourse.tile as tile

    nc = bacc.Bacc("TRN2", target_bir_lowering=False, debug=False,
                   num_devices=NCORES)
    with tile.TileContext(nc) as tc:
        with ExitStack() as ctx:
            _build(nc, tc, ctx)
    nc.compile()
    _CACHE["nc"] = nc
    return nc


def kernel(**inputs):
    from concourse.bass_utils import run_bass_kernel_spmd

    shared, xs = _pack_inputs(
        inputs["x"], inputs["Wproj"], inputs["attn_norm"], inputs["Wq"],
        inputs["Wk"], inputs["Wv"], inputs["Wo"], inputs["ffn_norm"],
        inputs["W1"], inputs["W2"], inputs["Wlm"])

    nc = _get_compiled()
    in_maps = [dict(shared, xs=xs[c]) for c in range(NCORES)]
    res = run_bass_kernel_spmd(nc, in_maps, core_ids=list(range(NCORES)),
                               trace=bool(int(os.environ.get("KERNEL_TRACE", "0"))))
    logits = np.concatenate([r["logits"] for r in res.results], axis=0)
    _CACHE["last_exec_ns"] = res.exec_time_ns
    return logits.reshape(B, S, S_C, V).astype(F32)


if __name__ == "__main__":
    nc = _get_compiled()
    print("built + compiled OK")
